# revision 26
# baseline (speedup 1.0000x reference)
"""Columnwise imputer (per-feature LSTM) Trainium2 kernel.

Problem: D=32 independent per-feature LSTMs (input D-1=31, hidden H=64),
B=128, T=128.  x_hat[b,t,d] = W_out[d] @ h_d(t) + b_out[d].

Sharding: expert-parallel over the D feature axis -- 4 features per core
(2 "pairs" of 2 features).  Each core runs its 4 LSTMs over the full
batch; the host gathers per-core [B, T, 4] outputs.

Device kernel structure (per core, per timestep t, per pair):
  - gates psum tile fp32; each [128,128] chunk: partitions 0-63 feature A
    of the pair, 64-127 feature B; free dim = batch.
  - per chunk: ip matmul (K=33: 32 features + ones row carrying the bias)
    accumulated with rec matmul (K=128: block-diag [h_A; h_B] weights).
  - ScalarE: sigmoid over i,f,o chunks, tanh over g, later tanh over c.
  - VectorE: t1 = i*g, t2 = f*c, c' = t1+t2, h' = o*tanh(c').
  - output head in-loop: y^T[b, feat] = h_t (stationary) @ W_out cols,
    N=2 matmul per pair into a persistent psum tile [128, 512].

The kernel is recurrence-latency-bound (wall ~= T x per-step chain), so the
default variant v5 minimizes the serial chain: sigma-trick (all activations
are a single Sigmoid table; tanh expressed via sigmoid with weight folding),
fused scalar_tensor_tensor cell ops, split sigmoid so the o-gate is off the
critical path, and software-pipelined emission (the s_c/h tail of step t-1
interleaves with the matmuls/sigmoid of step t on every engine queue).

Variants (env KV): v1 batched, v2 per-pair, v3 sigma-trick,
v4 rotated single-sigma, v5 rotated split-sigma (default),
v6 v5+two-bank psum split.

Cost-model (TimelineSim) predictions, T=128, per core:
  v1 591us / v2 427us / v3 442us / v4 421us / v5 397us / v6 394us.
Hardware-verified (8 axon trn2 cores): relative error 4.46e-3 vs the
float64 reference (bf16 matmul inputs + bf16 state dominate the error;
the recurrence is contractive so it does not compound).

Host prep: masking, transpose to [feat, t, b], weight packing (block-diag,
zero-diagonal full-D input weights, biases as 33rd row, sigma-trick weight
doubling), bias b_out added on the host.
"""

import os
import sys

import numpy as np

try:
    import concourse  # noqa: F401  (provided by the axon boot environment)
except ImportError:
    sys.path.insert(0, "/root/.axon_site/_ro/trn_rl_repo")

import ml_dtypes

D = 32
H = 64
B = 128
T = 128
NCORES = 8
DLOC = D // NCORES  # 4 features per core
NPAIR = DLOC // 2  # 2 pairs per core

VARIANT = os.environ.get("KV", "v5")
SDT_NAME = os.environ.get("SDT", "bf16")  # sigmoid/tanh output dtype
CDT_NAME = os.environ.get("CDT", "bf16")  # cell state dtype
USE_GPSIMD = bool(int(os.environ.get("GPS", "0")))  # offload f*c to GPSIMD

BF16 = ml_dtypes.bfloat16

# gate row ranges in the torch-stacked [4H] layout (i, f, g, o)
GATE_SLICES = {
    "i": slice(0 * H, 1 * H),
    "f": slice(1 * H, 2 * H),
    "g": slice(2 * H, 3 * H),
    "o": slice(3 * H, 4 * H),
}

# psum chunk order (gate, pair):
if VARIANT == "v1":
    CHUNK_DEFS = [(g, p) for g in ("i", "f", "o", "g") for p in range(NPAIR)]
elif VARIANT == "v2":
    CHUNK_DEFS = [(g, p) for p in range(NPAIR) for g in ("i", "f", "o", "g")]
else:  # v3/v4/v5 sigma-trick layouts
    CHUNK_DEFS = [(g, p) for p in range(NPAIR) for g in ("i", "g", "f", "o")]

# v3 sigma-trick weight folding:
#   - h is stored as h/2 (STT produces (sigma(2c)-0.5)*o), so all recurrent
#     and output weights that contract h are doubled.
#   - tanh(x) = 2*sigmoid(2x) - 1: g-gate logits are doubled so a single
#     sigmoid pass covers all four gates; the -1/x2 correction happens in
#     cheap DVE tensor_scalar/STT ops (or is folded into downstream weights).
V3 = VARIANT in ("v3", "v4", "v5", "v6")

_CACHE = {}


def _build_bass():
    """Build (and cache) the Bass module. Same program for all 8 cores."""
    if "nc" in _CACHE:
        return _CACHE["nc"]

    import concourse.bacc as bacc
    import concourse.mybir as mybir
    import concourse.tile as tile

    f32 = mybir.dt.float32
    bf16 = mybir.dt.bfloat16
    sdt = bf16 if SDT_NAME == "bf16" else f32
    cdt = bf16 if CDT_NAME == "bf16" else f32
    SIG = mybir.ActivationFunctionType.Sigmoid
    TANH = mybir.ActivationFunctionType.Tanh

    nc = bacc.Bacc("TRN2", target_bir_lowering=False, debug=False, num_devices=NCORES)

    xmT_d = nc.dram_tensor("xmT", [D + 1, T * B], bf16, kind="ExternalInput").ap()
    ipw_d = nc.dram_tensor("ipw", [D + 1, 8 * 128], bf16, kind="ExternalInput").ap()
    recw_d = nc.dram_tensor("recw", [128, 8 * 128], bf16, kind="ExternalInput").ap()
    outw_d = nc.dram_tensor("outw", [128, 2 * NPAIR], bf16, kind="ExternalInput").ap()
    y_d = nc.dram_tensor("y", [B, NPAIR * 2 * T], f32, kind="ExternalOutput").ap()

    with tile.TileContext(nc) as tc:
        with (
            tc.tile_pool(name="const", bufs=1) as const_pool,
            tc.tile_pool(name="psum_gates", bufs=int(os.environ.get("PGBUFS", "2")), space="PSUM") as pg_pool,
            tc.tile_pool(name="psum_y", bufs=1, space="PSUM") as py_pool,
            tc.tile_pool(name="sig", bufs=6) as sig_pool,
            tc.tile_pool(name="tan", bufs=6) as tan_pool,
            tc.tile_pool(name="tmp", bufs=6) as tmp_pool,
            tc.tile_pool(name="cst", bufs=6) as c_pool,
            tc.tile_pool(name="hst", bufs=6) as h_pool,
        ):
            xmT = const_pool.tile([D + 1, T * B], bf16)
            ipw = const_pool.tile([D + 1, 8 * 128], bf16)
            recw = const_pool.tile([128, 8 * 128], bf16)
            outw = const_pool.tile([128, 2 * NPAIR], bf16)
            nc.sync.dma_start(out=xmT, in_=xmT_d)
            nc.sync.dma_start(out=ipw, in_=ipw_d)
            nc.sync.dma_start(out=recw, in_=recw_d)
            nc.sync.dma_start(out=outw, in_=outw_d)

            y_ps = py_pool.tile([B, NPAIR * 2 * T], f32)

            if VARIANT == "v6":
                with tc.tile_pool(name="psum_b", bufs=1, space="PSUM") as pgb_pool:
                    _emit_v6(nc, tc, mybir, pg_pool, pgb_pool, sig_pool,
                             tan_pool, tmp_pool, c_pool, h_pool, xmT, ipw,
                             recw, outw, y_ps, f32, sdt, cdt, SIG, TANH)
            else:
                emit = {"v1": _emit_v1, "v2": _emit_v2, "v3": _emit_v3,
                        "v4": _emit_v4, "v5": _emit_v5}[VARIANT]
                emit(nc, tc, mybir, pg_pool, sig_pool, tan_pool, tmp_pool,
                     c_pool, h_pool, xmT, ipw, recw, outw, y_ps, f32, sdt, cdt,
                     SIG, TANH)

            y_sb = const_pool.tile([B, NPAIR * 2 * T], f32)
            nc.vector.tensor_copy(y_sb, y_ps)
            nc.sync.dma_start(out=y_d, in_=y_sb)

    nc.compile()
    _CACHE["nc"] = nc
    return nc


def _emit_v1(nc, tc, mybir, pg_pool, sig_pool, tan_pool, tmp_pool, c_pool,
             h_pool, xmT, ipw, recw, outw, y_ps, f32, sdt, cdt, SIG, TANH):
    """Batched layout: psum [i01|i23|f01|f23|o01|o23|g01|g23]."""
    h_prev = None
    c_prev = None
    for t in range(T):
        pg = pg_pool.tile([128, 8 * 128], f32, tag="pg")
        xm_t = xmT[:, t * B : (t + 1) * B]
        for ci in range(8):
            _, pair = CHUNK_DEFS[ci]
            sl = slice(ci * 128, (ci + 1) * 128)
            nc.tensor.matmul(pg[:, sl], ipw[:, sl], xm_t, start=True, stop=(t == 0))
            if t > 0:
                nc.tensor.matmul(
                    pg[:, sl], recw[:, sl], h_prev[:, pair, :],
                    start=False, stop=True,
                )

        ssig = sig_pool.tile([128, 6 * 128], sdt, tag="ssig")
        nc.scalar.activation(ssig, pg[:, 0 : 6 * 128], SIG)
        stan = tan_pool.tile([128, 2 * 128], sdt, tag="stan")
        nc.scalar.activation(stan, pg[:, 6 * 128 : 8 * 128], TANH)

        i_ap = ssig[:, 0:256]
        f_ap = ssig[:, 256:512]
        o_ap = ssig[:, 512:768]

        c_new = c_pool.tile([128, 2 * 128], cdt, tag="c")
        if t == 0:
            nc.vector.tensor_mul(c_new, i_ap, stan)
        else:
            t1 = tmp_pool.tile([128, 2 * 128], sdt, tag="t1")
            nc.vector.tensor_mul(t1, i_ap, stan)
            t2 = tmp_pool.tile([128, 2 * 128], cdt, tag="t2")
            nc.vector.tensor_mul(t2, f_ap, c_prev)
            nc.vector.tensor_add(c_new, t1, t2)

        sc = tan_pool.tile([128, 2 * 128], sdt, tag="sc")
        nc.scalar.activation(sc, c_new, TANH)

        h_new = h_pool.tile([128, NPAIR, B], mybir.dt.bfloat16, tag="h")
        nc.vector.tensor_mul(
            h_new,
            o_ap.rearrange("p (q b) -> p q b", q=NPAIR),
            sc.rearrange("p (q b) -> p q b", q=NPAIR),
        )

        for pair in range(NPAIR):
            nc.tensor.matmul(
                y_ps[:, pair * 2 * T + 2 * t : pair * 2 * T + 2 * t + 2],
                h_new[:, pair, :],
                outw[:, 2 * pair : 2 * pair + 2],
                start=True, stop=True,
            )

        h_prev = h_new
        c_prev = c_new


def _emit_v2(nc, tc, mybir, pg_pool, sig_pool, tan_pool, tmp_pool, c_pool,
             h_pool, xmT, ipw, recw, outw, y_ps, f32, sdt, cdt, SIG, TANH):
    """Per-pair chains: psum per (t, pair) = [i|f|o|g], chunks at
    ipw/recw columns (pair*4 + k)*128."""
    h_prev = [None] * NPAIR
    c_prev = [None] * NPAIR
    for t in range(T):
        xm_t = xmT[:, t * B : (t + 1) * B]
        for pair in range(NPAIR):
            pg = pg_pool.tile([128, 4 * 128], f32, tag=f"pg{pair}")
            for k in range(4):
                ci = pair * 4 + k
                wsl = slice(ci * 128, (ci + 1) * 128)
                psl = slice(k * 128, (k + 1) * 128)
                nc.tensor.matmul(
                    pg[:, psl], ipw[:, wsl], xm_t, start=True, stop=(t == 0)
                )
                if t > 0:
                    nc.tensor.matmul(
                        pg[:, psl], recw[:, wsl], h_prev[pair],
                        start=False, stop=True,
                    )

            ssig = sig_pool.tile([128, 3 * 128], sdt, tag=f"ssig{pair}")
            nc.scalar.activation(ssig, pg[:, 0 : 3 * 128], SIG)
            stan = tan_pool.tile([128, 128], sdt, tag=f"stan{pair}")
            nc.scalar.activation(stan, pg[:, 3 * 128 : 4 * 128], TANH)

            i_ap = ssig[:, 0:128]
            f_ap = ssig[:, 128:256]
            o_ap = ssig[:, 256:384]

            c_new = c_pool.tile([128, 128], cdt, tag=f"c{pair}")
            if t == 0:
                nc.vector.tensor_mul(c_new, i_ap, stan)
            else:
                t1 = tmp_pool.tile([128, 128], sdt, tag=f"t1{pair}")
                nc.vector.tensor_mul(t1, i_ap, stan)
                t2 = tmp_pool.tile([128, 128], cdt, tag=f"t2{pair}")
                nc.vector.tensor_mul(t2, f_ap, c_prev[pair])
                nc.vector.tensor_add(c_new, t1, t2)

            sc = tan_pool.tile([128, 128], sdt, tag=f"sc{pair}")
            nc.scalar.activation(sc, c_new, TANH)

            h_new = h_pool.tile([128, B], mybir.dt.bfloat16, tag=f"h{pair}")
            nc.vector.tensor_mul(h_new, o_ap, sc)

            nc.tensor.matmul(
                y_ps[:, pair * 2 * T + 2 * t : pair * 2 * T + 2 * t + 2],
                h_new,
                outw[:, 2 * pair : 2 * pair + 2],
                start=True, stop=True,
            )

            h_prev[pair] = h_new
            c_prev[pair] = c_new


def _emit_v3(nc, tc, mybir, pg_pool, sig_pool, tan_pool, tmp_pool, c_pool,
             h_pool, xmT, ipw, recw, outw, y_ps, f32, sdt, cdt, SIG, TANH):
    """Sigma-trick + chain-latency-optimized emission.

    Chunk order per pair is [i, g, f, o] (see CHUNK_DEFS).  Per pair per t:
      sigma1 = sigmoid(pg[i,g])   (after only the first 4 matmuls)
      t1'   = (s_g - 0.5) * s_i   (STT; == i*g/2)
      sigma2 = sigmoid(pg[f,o])   (off critical path, overlaps DVE)
      t2    = s_f * c_prev        (TT)
      c     = 2*t1' + t2          (STT)
      s_c   = sigmoid(2c) fp32
      h2    = (s_c - 0.5) * s_o   (STT; == h/2, x2 folded into recw/outw)
    """
    ALU = mybir.AluOpType
    h_prev = [None] * NPAIR
    c_prev = [None] * NPAIR
    y_mm = [None] * NPAIR  # deferred y matmul args from previous t

    for t in range(T):
        xm_t = xmT[:, t * B : (t + 1) * B]
        pgs = [None] * NPAIR

        # 1) gate matmuls for t: per chunk [rec(start), ip(stop)] -- PSUM
        # allows only one open accumulation group per bank.
        for pair in range(NPAIR):
            pg = pgs[pair] = pg_pool.tile(
                [128, 4 * 128], f32, tag=f"pg{pair}", name=f"pg{pair}"
            )
            for k in range(4):
                ci = pair * 4 + k
                sl = slice(k * 128, (k + 1) * 128)
                wsl = slice(ci * 128, (ci + 1) * 128)
                if t > 0:
                    nc.tensor.matmul(
                        pg[:, sl], recw[:, wsl], h_prev[pair],
                        start=True, stop=False,
                    )
                nc.tensor.matmul(
                    pg[:, sl], ipw[:, wsl], xm_t,
                    start=(t == 0), stop=True,
                )
        # y matmuls for t-1 (PE; operands long ready)
        for pair in range(NPAIR):
            if y_mm[pair] is not None:
                out_sl, h_tile = y_mm[pair]
                nc.tensor.matmul(
                    y_ps[:, out_sl], h_tile, outw[:, 2 * pair : 2 * pair + 2],
                    start=True, stop=True,
                )
                y_mm[pair] = None

        # 2) sigma1 over [i, g] chunks (critical path: t1')
        sig1 = []
        for pair in range(NPAIR):
            s1 = sig_pool.tile([128, 2 * 128], sdt, tag=f"s1{pair}")
            nc.scalar.activation(s1, pgs[pair][:, 0:256], SIG)
            sig1.append(s1)

        # 3) t1' = (s_g - 0.5) * s_i  == i*g/2   (STT)
        t1s = []
        for pair in range(NPAIR):
            t1 = tmp_pool.tile([128, 128], sdt, tag=f"t1{pair}")
            nc.vector.scalar_tensor_tensor(
                t1, sig1[pair][:, 128:256], 0.5, sig1[pair][:, 0:128],
                ALU.subtract, ALU.mult,
            )
            t1s.append(t1)

        # 4) sigma2 over [f, o] chunks (overlaps DVE work above)
        sig2 = []
        for pair in range(NPAIR):
            s2 = sig_pool.tile([128, 2 * 128], sdt, tag=f"s2{pair}")
            nc.scalar.activation(s2, pgs[pair][:, 256:512], SIG)
            sig2.append(s2)

        # 5) t2 = f * c_prev ;  6) c = 2*t1' + t2
        for pair in range(NPAIR):
            c_new = c_pool.tile([128, 128], cdt, tag=f"c{pair}")
            if t == 0:
                nc.vector.tensor_scalar_mul(c_new, t1s[pair], 2.0)
            else:
                t2 = tmp_pool.tile([128, 128], cdt, tag=f"t2{pair}")
                eng = nc.gpsimd if USE_GPSIMD else nc.vector
                eng.tensor_mul(t2, sig2[pair][:, 0:128], c_prev[pair])
                nc.vector.scalar_tensor_tensor(
                    c_new, t1s[pair], 2.0, t2, ALU.mult, ALU.add
                )
            c_prev[pair] = c_new

        # 7) s_c = sigmoid(2c)  (fp32 out: avoids cancellation in s_c-0.5)
        # 8) h2 = (s_c - 0.5) * o   == h/2
        for pair in range(NPAIR):
            sc = tan_pool.tile([128, 128], f32, tag=f"sc{pair}")
            nc.scalar.activation(sc, c_prev[pair], SIG, scale=2.0)
            h2 = h_pool.tile([128, B], mybir.dt.bfloat16, tag=f"h{pair}")
            nc.vector.scalar_tensor_tensor(
                h2, sc, 0.5, sig2[pair][:, 128:256],
                ALU.subtract, ALU.mult,
            )
            h_prev[pair] = h2
            y_mm[pair] = (
                slice(pair * 2 * T + 2 * t, pair * 2 * T + 2 * t + 2),
                h2,
            )

    # trailing y matmuls for t = T-1
    for pair in range(NPAIR):
        out_sl, h_tile = y_mm[pair]
        nc.tensor.matmul(
            y_ps[:, out_sl], h_tile, outw[:, 2 * pair : 2 * pair + 2],
            start=True, stop=True,
        )


def _emit_v4(*args, **kw):
    _emit_rotated(*args, split_sigma=False, **kw)


def _emit_v5(*args, **kw):
    _emit_rotated(*args, split_sigma=True, **kw)


def _emit_rotated(nc, tc, mybir, pg_pool, sig_pool, tan_pool, tmp_pool, c_pool,
                  h_pool, xmT, ipw, recw, outw, y_ps, f32, sdt, cdt, SIG, TANH,
                  split_sigma=False):
    """Software-pipelined emission: iteration tau emits the *tail* of step
    tau-1 (s_c, h2) before the matmuls/sigmoid/cell ops of step tau, so each
    engine's in-order queue cycles through both pair-chains with the tail of
    one step overlapping the head of the next.

    Chunk order [i, g, f, o].  split_sigma: sigma1=[i,g,f], sigma2=[o]
    (sigma2 is only needed by h2 one iteration later)."""
    ALU = mybir.AluOpType
    h_prev = [None] * NPAIR
    c_prev = [None] * NPAIR
    o_src = [None] * NPAIR  # AP of sigmoid(o) for the h2 of the previous step
    y_mm = [None] * NPAIR

    for t in range(T + 1):
        # ---- tail of step t-1: s_c, h2 ----
        if t > 0:
            scs = []
            for pair in range(NPAIR):
                sc = tan_pool.tile([128, 128], f32, tag=f"sc{pair}", name=f"sc{pair}")
                nc.scalar.activation(sc, c_prev[pair], SIG, scale=2.0)
                scs.append(sc)
            for pair in range(NPAIR):
                h2 = h_pool.tile([128, B], mybir.dt.bfloat16, tag=f"h{pair}",
                                 name=f"h{pair}")
                nc.vector.scalar_tensor_tensor(
                    h2, scs[pair], 0.5, o_src[pair], ALU.subtract, ALU.mult
                )
                h_prev[pair] = h2
                y_mm[pair] = (
                    slice(pair * 2 * T + 2 * (t - 1), pair * 2 * T + 2 * (t - 1) + 2),
                    h2,
                )
        if t == T:
            break

        xm_t = xmT[:, t * B : (t + 1) * B]

        # ---- gate matmuls for t ----
        pgs = []
        for pair in range(NPAIR):
            pg = pg_pool.tile([128, 4 * 128], f32, tag=f"pg{pair}", name=f"pg{pair}")
            pgs.append(pg)
            for k in range(4):
                ci = pair * 4 + k
                sl = slice(k * 128, (k + 1) * 128)
                wsl = slice(ci * 128, (ci + 1) * 128)
                if t > 0:
                    nc.tensor.matmul(
                        pg[:, sl], recw[:, wsl], h_prev[pair],
                        start=True, stop=False,
                    )
                nc.tensor.matmul(
                    pg[:, sl], ipw[:, wsl], xm_t, start=(t == 0), stop=True
                )
        # y matmuls for t-1
        for pair in range(NPAIR):
            if y_mm[pair] is not None:
                out_sl, h_tile = y_mm[pair]
                nc.tensor.matmul(
                    y_ps[:, out_sl], h_tile, outw[:, 2 * pair : 2 * pair + 2],
                    start=True, stop=True,
                )
                y_mm[pair] = None

        # ---- sigmoid(s) for t ----
        sigs = []
        if split_sigma:
            for pair in range(NPAIR):
                s1 = sig_pool.tile([128, 3 * 128], sdt, tag=f"s1{pair}",
                                   name=f"s1{pair}")
                nc.scalar.activation(s1, pgs[pair][:, 0:384], SIG)
                sigs.append(s1)
            for pair in range(NPAIR):
                s2 = sig_pool.tile([128, 128], sdt, tag=f"s2{pair}",
                                   name=f"s2{pair}")
                nc.scalar.activation(s2, pgs[pair][:, 384:512], SIG)
                o_src[pair] = s2
        else:
            for pair in range(NPAIR):
                s = sig_pool.tile([128, 4 * 128], sdt, tag=f"s{pair}",
                                  name=f"s{pair}")
                nc.scalar.activation(s, pgs[pair], SIG)
                sigs.append(s)
                o_src[pair] = s[:, 384:512]

        # ---- cell update for t: t1' = (s_g-0.5)*s_i ; c = 2*t1' + f*c ----
        for pair in range(NPAIR):
            s = sigs[pair]
            t1 = tmp_pool.tile([128, 128], sdt, tag=f"t1{pair}", name=f"t1{pair}")
            nc.vector.scalar_tensor_tensor(
                t1, s[:, 128:256], 0.5, s[:, 0:128], ALU.subtract, ALU.mult
            )
            c_new = c_pool.tile([128, 128], cdt, tag=f"c{pair}", name=f"c{pair}")
            if t == 0:
                nc.vector.tensor_scalar_mul(c_new, t1, 2.0)
            else:
                t2 = tmp_pool.tile([128, 128], cdt, tag=f"t2{pair}", name=f"t2{pair}")
                eng = nc.gpsimd if USE_GPSIMD else nc.vector
                eng.tensor_mul(t2, s[:, 256:384], c_prev[pair])
                nc.vector.scalar_tensor_tensor(
                    c_new, t1, 2.0, t2, ALU.mult, ALU.add
                )
            c_prev[pair] = c_new

    # trailing y matmuls for t = T-1
    for pair in range(NPAIR):
        out_sl, h_tile = y_mm[pair]
        nc.tensor.matmul(
            y_ps[:, out_sl], h_tile, outw[:, 2 * pair : 2 * pair + 2],
            start=True, stop=True,
        )


def _emit_v6(nc, tc, mybir, pga_pool, pgb_pool, sig_pool, tan_pool, tmp_pool,
             c_pool, h_pool, xmT, ipw, recw, outw, y_ps, f32, sdt, cdt,
             SIG, TANH):
    """v5 + two-bank psum split per pair: pga=[i,g] (bufs=2), pgb=[f,o]
    (bufs=1), so sigma1 only waits on 4 matmuls and is smaller."""
    ALU = mybir.AluOpType
    h_prev = [None] * NPAIR
    c_prev = [None] * NPAIR
    o_src = [None] * NPAIR
    y_mm = [None] * NPAIR

    for t in range(T + 1):
        if t > 0:
            scs = []
            for pair in range(NPAIR):
                sc = tan_pool.tile([128, 128], f32, tag=f"sc{pair}", name=f"sc{pair}")
                nc.scalar.activation(sc, c_prev[pair], SIG, scale=2.0)
                scs.append(sc)
            for pair in range(NPAIR):
                h2 = h_pool.tile([128, B], mybir.dt.bfloat16, tag=f"h{pair}",
                                 name=f"h{pair}")
                nc.vector.scalar_tensor_tensor(
                    h2, scs[pair], 0.5, o_src[pair], ALU.subtract, ALU.mult
                )
                h_prev[pair] = h2
                y_mm[pair] = (
                    slice(pair * 2 * T + 2 * (t - 1), pair * 2 * T + 2 * (t - 1) + 2),
                    h2,
                )
        if t == T:
            break

        xm_t = xmT[:, t * B : (t + 1) * B]

        # gate matmuls: [i,g] chunks (pga) for both pairs first, then [f,o]
        pgas, pgbs = [], []
        for pair in range(NPAIR):
            pga = pga_pool.tile([128, 2 * 128], f32, tag=f"pga{pair}",
                                name=f"pga{pair}")
            pgas.append(pga)
            for k in range(2):
                ci = pair * 4 + k
                sl = slice(k * 128, (k + 1) * 128)
                wsl = slice(ci * 128, (ci + 1) * 128)
                if t > 0:
                    nc.tensor.matmul(pga[:, sl], recw[:, wsl], h_prev[pair],
                                     start=True, stop=False)
                nc.tensor.matmul(pga[:, sl], ipw[:, wsl], xm_t,
                                 start=(t == 0), stop=True)
        for pair in range(NPAIR):
            pgb = pgb_pool.tile([128, 2 * 128], f32, tag=f"pgb{pair}",
                                name=f"pgb{pair}")
            pgbs.append(pgb)
            for k in range(2, 4):
                ci = pair * 4 + k
                sl = slice((k - 2) * 128, (k - 1) * 128)
                wsl = slice(ci * 128, (ci + 1) * 128)
                if t > 0:
                    nc.tensor.matmul(pgb[:, sl], recw[:, wsl], h_prev[pair],
                                     start=True, stop=False)
                nc.tensor.matmul(pgb[:, sl], ipw[:, wsl], xm_t,
                                 start=(t == 0), stop=True)
        for pair in range(NPAIR):
            if y_mm[pair] is not None:
                out_sl, h_tile = y_mm[pair]
                nc.tensor.matmul(
                    y_ps[:, out_sl], h_tile, outw[:, 2 * pair : 2 * pair + 2],
                    start=True, stop=True,
                )
                y_mm[pair] = None

        # sigma1 = [i, g]; sigma2 = [f, o]
        s1s, s2s = [], []
        for pair in range(NPAIR):
            s1 = sig_pool.tile([128, 2 * 128], sdt, tag=f"s1{pair}",
                               name=f"s1{pair}")
            nc.scalar.activation(s1, pgas[pair], SIG)
            s1s.append(s1)
        for pair in range(NPAIR):
            s2 = sig_pool.tile([128, 2 * 128], sdt, tag=f"s2{pair}",
                               name=f"s2{pair}")
            nc.scalar.activation(s2, pgbs[pair], SIG)
            s2s.append(s2)
            o_src[pair] = s2[:, 128:256]

        # t1' = (s_g - 0.5)*s_i ; c = 2*t1' + s_f*c_prev
        for pair in range(NPAIR):
            s1 = s1s[pair]
            t1 = tmp_pool.tile([128, 128], sdt, tag=f"t1{pair}", name=f"t1{pair}")
            nc.vector.scalar_tensor_tensor(
                t1, s1[:, 128:256], 0.5, s1[:, 0:128], ALU.subtract, ALU.mult
            )
            c_new = c_pool.tile([128, 128], cdt, tag=f"c{pair}", name=f"c{pair}")
            if t == 0:
                nc.vector.tensor_scalar_mul(c_new, t1, 2.0)
            else:
                t2 = tmp_pool.tile([128, 128], cdt, tag=f"t2{pair}", name=f"t2{pair}")
                nc.vector.tensor_mul(t2, s2s[pair][:, 0:128], c_prev[pair])
                nc.vector.scalar_tensor_tensor(
                    c_new, t1, 2.0, t2, ALU.mult, ALU.add
                )
            c_prev[pair] = c_new

    for pair in range(NPAIR):
        out_sl, h_tile = y_mm[pair]
        nc.tensor.matmul(
            y_ps[:, out_sl], h_tile, outw[:, 2 * pair : 2 * pair + 2],
            start=True, stop=True,
        )


def _w_full(W_ih_d, d):
    """[4H, D-1] -> [4H, D] with column d zero and the 'other feature'
    columns scattered back to their true feature index."""
    out = np.zeros((4 * H, D), np.float32)
    idx = [j for j in range(D) if j != d]
    out[:, idx] = W_ih_d
    return out


def _pack_core_inputs(core, xmT_np, W_ih, W_hh, b_ih, b_hh, W_out):
    """Pack weights for one core (features 4*core .. 4*core+3)."""
    ipw = np.zeros((D + 1, 8 * 128), np.float32)
    recw = np.zeros((128, 8 * 128), np.float32)
    outw = np.zeros((128, 2 * NPAIR), np.float32)

    for ci, (gate, pair) in enumerate(CHUNK_DEFS):
        gs = GATE_SLICES[gate]
        for half in range(2):
            d = DLOC * core + 2 * pair + half
            cols = slice(ci * 128 + 64 * half, ci * 128 + 64 * half + 64)
            rows = slice(64 * half, 64 * half + 64)
            # rec: block-diag W_hh[d, gate_rows, :].T  ([K=h, M=gate_row])
            recw[rows, cols] = W_hh[d, gs, :].T
            # ip: full-D input weights with zero self-column, bias in row 32
            wf = _w_full(W_ih[d], d)  # [4H, D]
            ipw[0:D, cols] = wf[gs, :].T
            ipw[D, cols] = b_ih[d, gs] + b_hh[d, gs]

    for pair in range(NPAIR):
        for half in range(2):
            d = DLOC * core + 2 * pair + half
            outw[64 * half : 64 * half + 64, 2 * pair + half] = W_out[d]

    if V3:
        # sigma-trick folding: h is stored as h/2 -> double recw/outw;
        # g-gate logits doubled -> double g chunks of ipw and recw again.
        recw *= 2.0
        outw *= 2.0
        for ci, (gate, _) in enumerate(CHUNK_DEFS):
            if gate == "g":
                ipw[:, ci * 128 : (ci + 1) * 128] *= 2.0
                recw[:, ci * 128 : (ci + 1) * 128] *= 2.0

    return {
        "xmT": xmT_np,
        "ipw": ipw.astype(BF16),
        "recw": recw.astype(BF16),
        "outw": outw.astype(BF16),
    }


def _prep_in_maps(x_raw, mask_pad, W_ih, W_hh, b_ih, b_hh, W_out):
    xm = np.where(mask_pad[:, :, None], x_raw, 0.0).astype(np.float32)  # [B,T,D]
    xmT = np.empty((D + 1, T * B), np.float32)
    xmT[0:D] = xm.transpose(2, 1, 0).reshape(D, T * B)  # [d, t*B + b]
    xmT[D] = 1.0
    xmT_np = xmT.astype(BF16)
    return [
        _pack_core_inputs(k, xmT_np, W_ih, W_hh, b_ih, b_hh, W_out)
        for k in range(NCORES)
    ]


def _assemble_output(results, b_out):
    """results[k]["y"]: [B, NPAIR*2*T] fp32, layout [b, pair*2T + t*2 + half]."""
    x_hat = np.empty((B, T, D), np.float32)
    for k in range(NCORES):
        y = np.asarray(results[k]["y"]).reshape(B, NPAIR, T, 2)
        for pair in range(NPAIR):
            for half in range(2):
                d = DLOC * k + 2 * pair + half
                x_hat[:, :, d] = y[:, pair, :, half] + b_out[d]
    return x_hat


def kernel(x_raw, mask_pad, W_ih, W_hh, b_ih, b_hh, W_out, b_out):
    x_raw = np.asarray(x_raw, np.float32)
    mask_pad = np.asarray(mask_pad)
    W_ih = np.asarray(W_ih, np.float32)
    W_hh = np.asarray(W_hh, np.float32)
    b_ih = np.asarray(b_ih, np.float32)
    b_hh = np.asarray(b_hh, np.float32)
    W_out = np.asarray(W_out, np.float32)
    b_out = np.asarray(b_out, np.float32)

    from concourse import bass_utils

    nc = _build_bass()
    in_maps = _prep_in_maps(x_raw, mask_pad, W_ih, W_hh, b_ih, b_hh, W_out)
    res = bass_utils.run_bass_kernel_spmd(
        nc,
        in_maps,
        core_ids=list(range(NCORES)),
        trace=bool(int(os.environ.get("KERNEL_TRACE", "0"))),
    )
    _CACHE["last_results"] = res
    return _assemble_output(res.results, b_out)


# revision 33
# speedup vs baseline: 1.0116x; 1.0116x over previous
"""Columnwise imputer (per-feature LSTM) Trainium2 kernel.

Problem: D=32 independent per-feature LSTMs (input D-1=31, hidden H=64),
B=128, T=128.  x_hat[b,t,d] = W_out[d] @ h_d(t) + b_out[d].

Sharding: expert-parallel over the D feature axis -- 4 features per core
(2 "pairs" of 2 features).  Each core runs its 4 LSTMs over the full
batch; the host gathers per-core [B, T, 4] outputs.

Device kernel structure (per core, per timestep t, per pair):
  - gates psum tile fp32; each [128,128] chunk: partitions 0-63 feature A
    of the pair, 64-127 feature B; free dim = batch.
  - per chunk: ip matmul (K=33: 32 features + ones row carrying the bias)
    accumulated with rec matmul (K=128: block-diag [h_A; h_B] weights).
  - ScalarE: sigmoid over i,f,o chunks, tanh over g, later tanh over c.
  - VectorE: t1 = i*g, t2 = f*c, c' = t1+t2, h' = o*tanh(c').
  - output head in-loop: y^T[b, feat] = h_t (stationary) @ W_out cols,
    N=2 matmul per pair into a persistent psum tile [128, 512].

The kernel is recurrence-latency-bound (wall ~= T x per-step chain), so the
default variant v5 minimizes the serial chain: sigma-trick (all activations
are a single Sigmoid table; tanh expressed via sigmoid with weight folding),
fused scalar_tensor_tensor cell ops, split sigmoid so the o-gate is off the
critical path, and software-pipelined emission (the s_c/h tail of step t-1
interleaves with the matmuls/sigmoid of step t on every engine queue).

Variants (env KV): v1 batched, v2 per-pair, v3 sigma-trick,
v4 rotated single-sigma, v5 rotated split-sigma (default),
v6 v5+two-bank psum split.

Cost-model (TimelineSim) predictions, T=128, per core:
  v1 591us / v2 427us / v3 442us / v4 421us / v5 397us / v6 394us.
Hardware-verified (8 axon trn2 cores): relative error 4.46e-3 vs the
float64 reference (bf16 matmul inputs + bf16 state dominate the error;
the recurrence is contractive so it does not compound).

Host prep: masking, transpose to [feat, t, b], weight packing (block-diag,
zero-diagonal full-D input weights, biases as 33rd row, sigma-trick weight
doubling), bias b_out added on the host.
"""

import os
import sys

import numpy as np

try:
    import concourse  # noqa: F401  (provided by the axon boot environment)
except ImportError:
    sys.path.insert(0, "/root/.axon_site/_ro/trn_rl_repo")

import ml_dtypes

D = 32
H = 64
B = 128
T = 128
NCORES = 8
DLOC = D // NCORES  # 4 features per core
NPAIR = DLOC // 2  # 2 pairs per core

VARIANT = os.environ.get("KV", "v5")
SDT_NAME = os.environ.get("SDT", "bf16")  # sigmoid/tanh output dtype
CDT_NAME = os.environ.get("CDT", "bf16")  # cell state dtype
USE_GPSIMD = bool(int(os.environ.get("GPS", "0")))  # offload f*c to GPSIMD

BF16 = ml_dtypes.bfloat16

# gate row ranges in the torch-stacked [4H] layout (i, f, g, o)
GATE_SLICES = {
    "i": slice(0 * H, 1 * H),
    "f": slice(1 * H, 2 * H),
    "g": slice(2 * H, 3 * H),
    "o": slice(3 * H, 4 * H),
}

# psum chunk order (gate, pair):
if VARIANT == "v1":
    CHUNK_DEFS = [(g, p) for g in ("i", "f", "o", "g") for p in range(NPAIR)]
elif VARIANT == "v2":
    CHUNK_DEFS = [(g, p) for p in range(NPAIR) for g in ("i", "f", "o", "g")]
else:  # v3/v4/v5 sigma-trick layouts
    CHUNK_DEFS = [(g, p) for p in range(NPAIR) for g in ("i", "g", "f", "o")]

# v3 sigma-trick weight folding:
#   - h is stored as h/2 (STT produces (sigma(2c)-0.5)*o), so all recurrent
#     and output weights that contract h are doubled.
#   - tanh(x) = 2*sigmoid(2x) - 1: g-gate logits are doubled so a single
#     sigmoid pass covers all four gates; the -1/x2 correction happens in
#     cheap DVE tensor_scalar/STT ops (or is folded into downstream weights).
V3 = VARIANT in ("v3", "v4", "v5", "v6", "v7", "v8")

_CACHE = {}


def _build_bass():
    """Build (and cache) the Bass module. Same program for all 8 cores."""
    if "nc" in _CACHE:
        return _CACHE["nc"]

    import concourse.bacc as bacc
    import concourse.mybir as mybir
    import concourse.tile as tile

    f32 = mybir.dt.float32
    bf16 = mybir.dt.bfloat16
    sdt = bf16 if SDT_NAME == "bf16" else f32
    cdt = bf16 if CDT_NAME == "bf16" else f32
    SIG = mybir.ActivationFunctionType.Sigmoid
    TANH = mybir.ActivationFunctionType.Tanh

    nc = bacc.Bacc("TRN2", target_bir_lowering=False, debug=False, num_devices=NCORES)

    xmT_d = nc.dram_tensor("xmT", [D + 1, T * B], bf16, kind="ExternalInput").ap()
    ipw_d = nc.dram_tensor("ipw", [D + 1, 8 * 128], bf16, kind="ExternalInput").ap()
    recw_d = nc.dram_tensor("recw", [128, 8 * 128], bf16, kind="ExternalInput").ap()
    outw_d = nc.dram_tensor("outw", [128, 2 * NPAIR], bf16, kind="ExternalInput").ap()
    y_d = nc.dram_tensor("y", [B, NPAIR * 2 * T], f32, kind="ExternalOutput").ap()

    with tile.TileContext(nc) as tc:
        with (
            tc.tile_pool(name="const", bufs=1) as const_pool,
            tc.tile_pool(name="psum_gates", bufs=int(os.environ.get("PGBUFS", "2")), space="PSUM") as pg_pool,
            tc.tile_pool(name="psum_y", bufs=1, space="PSUM") as py_pool,
            tc.tile_pool(name="sig", bufs=6) as sig_pool,
            tc.tile_pool(name="tan", bufs=6) as tan_pool,
            tc.tile_pool(name="tmp", bufs=6) as tmp_pool,
            tc.tile_pool(name="cst", bufs=6) as c_pool,
            tc.tile_pool(name="hst", bufs=6) as h_pool,
        ):
            xmT = const_pool.tile([D + 1, T * B], bf16)
            ipw = const_pool.tile([D + 1, 8 * 128], bf16)
            recw = const_pool.tile([128, 8 * 128], bf16)
            outw = const_pool.tile([128, 2 * NPAIR], bf16)
            nc.sync.dma_start(out=xmT, in_=xmT_d)
            nc.sync.dma_start(out=ipw, in_=ipw_d)
            nc.sync.dma_start(out=recw, in_=recw_d)
            nc.sync.dma_start(out=outw, in_=outw_d)

            y_ps = py_pool.tile([B, NPAIR * 2 * T], f32)

            if VARIANT in ("v6", "v8"):
                with tc.tile_pool(name="psum_b", bufs=1, space="PSUM") as pgb_pool:
                    emit2 = _emit_v6 if VARIANT == "v6" else _emit_v8
                    emit2(nc, tc, mybir, pg_pool, pgb_pool, sig_pool,
                          tan_pool, tmp_pool, c_pool, h_pool, xmT, ipw,
                          recw, outw, y_ps, f32, sdt, cdt, SIG, TANH)
            else:
                emit = {"v1": _emit_v1, "v2": _emit_v2, "v3": _emit_v3,
                        "v4": _emit_v4, "v5": _emit_v5, "v7": _emit_v7}[VARIANT]
                emit(nc, tc, mybir, pg_pool, sig_pool, tan_pool, tmp_pool,
                     c_pool, h_pool, xmT, ipw, recw, outw, y_ps, f32, sdt, cdt,
                     SIG, TANH)

            y_sb = const_pool.tile([B, NPAIR * 2 * T], f32)
            nc.vector.tensor_copy(y_sb, y_ps)
            nc.sync.dma_start(out=y_d, in_=y_sb)

    nc.compile()
    _CACHE["nc"] = nc
    return nc


def _emit_v1(nc, tc, mybir, pg_pool, sig_pool, tan_pool, tmp_pool, c_pool,
             h_pool, xmT, ipw, recw, outw, y_ps, f32, sdt, cdt, SIG, TANH):
    """Batched layout: psum [i01|i23|f01|f23|o01|o23|g01|g23]."""
    h_prev = None
    c_prev = None
    for t in range(T):
        pg = pg_pool.tile([128, 8 * 128], f32, tag="pg")
        xm_t = xmT[:, t * B : (t + 1) * B]
        for ci in range(8):
            _, pair = CHUNK_DEFS[ci]
            sl = slice(ci * 128, (ci + 1) * 128)
            nc.tensor.matmul(pg[:, sl], ipw[:, sl], xm_t, start=True, stop=(t == 0))
            if t > 0:
                nc.tensor.matmul(
                    pg[:, sl], recw[:, sl], h_prev[:, pair, :],
                    start=False, stop=True,
                )

        ssig = sig_pool.tile([128, 6 * 128], sdt, tag="ssig")
        nc.scalar.activation(ssig, pg[:, 0 : 6 * 128], SIG)
        stan = tan_pool.tile([128, 2 * 128], sdt, tag="stan")
        nc.scalar.activation(stan, pg[:, 6 * 128 : 8 * 128], TANH)

        i_ap = ssig[:, 0:256]
        f_ap = ssig[:, 256:512]
        o_ap = ssig[:, 512:768]

        c_new = c_pool.tile([128, 2 * 128], cdt, tag="c")
        if t == 0:
            nc.vector.tensor_mul(c_new, i_ap, stan)
        else:
            t1 = tmp_pool.tile([128, 2 * 128], sdt, tag="t1")
            nc.vector.tensor_mul(t1, i_ap, stan)
            t2 = tmp_pool.tile([128, 2 * 128], cdt, tag="t2")
            nc.vector.tensor_mul(t2, f_ap, c_prev)
            nc.vector.tensor_add(c_new, t1, t2)

        sc = tan_pool.tile([128, 2 * 128], sdt, tag="sc")
        nc.scalar.activation(sc, c_new, TANH)

        h_new = h_pool.tile([128, NPAIR, B], mybir.dt.bfloat16, tag="h")
        nc.vector.tensor_mul(
            h_new,
            o_ap.rearrange("p (q b) -> p q b", q=NPAIR),
            sc.rearrange("p (q b) -> p q b", q=NPAIR),
        )

        for pair in range(NPAIR):
            nc.tensor.matmul(
                y_ps[:, pair * 2 * T + 2 * t : pair * 2 * T + 2 * t + 2],
                h_new[:, pair, :],
                outw[:, 2 * pair : 2 * pair + 2],
                start=True, stop=True,
            )

        h_prev = h_new
        c_prev = c_new


def _emit_v2(nc, tc, mybir, pg_pool, sig_pool, tan_pool, tmp_pool, c_pool,
             h_pool, xmT, ipw, recw, outw, y_ps, f32, sdt, cdt, SIG, TANH):
    """Per-pair chains: psum per (t, pair) = [i|f|o|g], chunks at
    ipw/recw columns (pair*4 + k)*128."""
    h_prev = [None] * NPAIR
    c_prev = [None] * NPAIR
    for t in range(T):
        xm_t = xmT[:, t * B : (t + 1) * B]
        for pair in range(NPAIR):
            pg = pg_pool.tile([128, 4 * 128], f32, tag=f"pg{pair}")
            for k in range(4):
                ci = pair * 4 + k
                wsl = slice(ci * 128, (ci + 1) * 128)
                psl = slice(k * 128, (k + 1) * 128)
                nc.tensor.matmul(
                    pg[:, psl], ipw[:, wsl], xm_t, start=True, stop=(t == 0)
                )
                if t > 0:
                    nc.tensor.matmul(
                        pg[:, psl], recw[:, wsl], h_prev[pair],
                        start=False, stop=True,
                    )

            ssig = sig_pool.tile([128, 3 * 128], sdt, tag=f"ssig{pair}")
            nc.scalar.activation(ssig, pg[:, 0 : 3 * 128], SIG)
            stan = tan_pool.tile([128, 128], sdt, tag=f"stan{pair}")
            nc.scalar.activation(stan, pg[:, 3 * 128 : 4 * 128], TANH)

            i_ap = ssig[:, 0:128]
            f_ap = ssig[:, 128:256]
            o_ap = ssig[:, 256:384]

            c_new = c_pool.tile([128, 128], cdt, tag=f"c{pair}")
            if t == 0:
                nc.vector.tensor_mul(c_new, i_ap, stan)
            else:
                t1 = tmp_pool.tile([128, 128], sdt, tag=f"t1{pair}")
                nc.vector.tensor_mul(t1, i_ap, stan)
                t2 = tmp_pool.tile([128, 128], cdt, tag=f"t2{pair}")
                nc.vector.tensor_mul(t2, f_ap, c_prev[pair])
                nc.vector.tensor_add(c_new, t1, t2)

            sc = tan_pool.tile([128, 128], sdt, tag=f"sc{pair}")
            nc.scalar.activation(sc, c_new, TANH)

            h_new = h_pool.tile([128, B], mybir.dt.bfloat16, tag=f"h{pair}")
            nc.vector.tensor_mul(h_new, o_ap, sc)

            nc.tensor.matmul(
                y_ps[:, pair * 2 * T + 2 * t : pair * 2 * T + 2 * t + 2],
                h_new,
                outw[:, 2 * pair : 2 * pair + 2],
                start=True, stop=True,
            )

            h_prev[pair] = h_new
            c_prev[pair] = c_new


def _emit_v3(nc, tc, mybir, pg_pool, sig_pool, tan_pool, tmp_pool, c_pool,
             h_pool, xmT, ipw, recw, outw, y_ps, f32, sdt, cdt, SIG, TANH):
    """Sigma-trick + chain-latency-optimized emission.

    Chunk order per pair is [i, g, f, o] (see CHUNK_DEFS).  Per pair per t:
      sigma1 = sigmoid(pg[i,g])   (after only the first 4 matmuls)
      t1'   = (s_g - 0.5) * s_i   (STT; == i*g/2)
      sigma2 = sigmoid(pg[f,o])   (off critical path, overlaps DVE)
      t2    = s_f * c_prev        (TT)
      c     = 2*t1' + t2          (STT)
      s_c   = sigmoid(2c) fp32
      h2    = (s_c - 0.5) * s_o   (STT; == h/2, x2 folded into recw/outw)
    """
    ALU = mybir.AluOpType
    h_prev = [None] * NPAIR
    c_prev = [None] * NPAIR
    y_mm = [None] * NPAIR  # deferred y matmul args from previous t

    for t in range(T):
        xm_t = xmT[:, t * B : (t + 1) * B]
        pgs = [None] * NPAIR

        # 1) gate matmuls for t: per chunk [rec(start), ip(stop)] -- PSUM
        # allows only one open accumulation group per bank.
        for pair in range(NPAIR):
            pg = pgs[pair] = pg_pool.tile(
                [128, 4 * 128], f32, tag=f"pg{pair}", name=f"pg{pair}"
            )
            for k in range(4):
                ci = pair * 4 + k
                sl = slice(k * 128, (k + 1) * 128)
                wsl = slice(ci * 128, (ci + 1) * 128)
                if t > 0:
                    nc.tensor.matmul(
                        pg[:, sl], recw[:, wsl], h_prev[pair],
                        start=True, stop=False,
                    )
                nc.tensor.matmul(
                    pg[:, sl], ipw[:, wsl], xm_t,
                    start=(t == 0), stop=True,
                )
        # y matmuls for t-1 (PE; operands long ready)
        for pair in range(NPAIR):
            if y_mm[pair] is not None:
                out_sl, h_tile = y_mm[pair]
                nc.tensor.matmul(
                    y_ps[:, out_sl], h_tile, outw[:, 2 * pair : 2 * pair + 2],
                    start=True, stop=True,
                )
                y_mm[pair] = None

        # 2) sigma1 over [i, g] chunks (critical path: t1')
        sig1 = []
        for pair in range(NPAIR):
            s1 = sig_pool.tile([128, 2 * 128], sdt, tag=f"s1{pair}")
            nc.scalar.activation(s1, pgs[pair][:, 0:256], SIG)
            sig1.append(s1)

        # 3) t1' = (s_g - 0.5) * s_i  == i*g/2   (STT)
        t1s = []
        for pair in range(NPAIR):
            t1 = tmp_pool.tile([128, 128], sdt, tag=f"t1{pair}")
            nc.vector.scalar_tensor_tensor(
                t1, sig1[pair][:, 128:256], 0.5, sig1[pair][:, 0:128],
                ALU.subtract, ALU.mult,
            )
            t1s.append(t1)

        # 4) sigma2 over [f, o] chunks (overlaps DVE work above)
        sig2 = []
        for pair in range(NPAIR):
            s2 = sig_pool.tile([128, 2 * 128], sdt, tag=f"s2{pair}")
            nc.scalar.activation(s2, pgs[pair][:, 256:512], SIG)
            sig2.append(s2)

        # 5) t2 = f * c_prev ;  6) c = 2*t1' + t2
        for pair in range(NPAIR):
            c_new = c_pool.tile([128, 128], cdt, tag=f"c{pair}")
            if t == 0:
                nc.vector.tensor_scalar_mul(c_new, t1s[pair], 2.0)
            else:
                t2 = tmp_pool.tile([128, 128], cdt, tag=f"t2{pair}")
                eng = nc.gpsimd if USE_GPSIMD else nc.vector
                eng.tensor_mul(t2, sig2[pair][:, 0:128], c_prev[pair])
                nc.vector.scalar_tensor_tensor(
                    c_new, t1s[pair], 2.0, t2, ALU.mult, ALU.add
                )
            c_prev[pair] = c_new

        # 7) s_c = sigmoid(2c)  (fp32 out: avoids cancellation in s_c-0.5)
        # 8) h2 = (s_c - 0.5) * o   == h/2
        for pair in range(NPAIR):
            sc = tan_pool.tile([128, 128], f32, tag=f"sc{pair}")
            nc.scalar.activation(sc, c_prev[pair], SIG, scale=2.0)
            h2 = h_pool.tile([128, B], mybir.dt.bfloat16, tag=f"h{pair}")
            nc.vector.scalar_tensor_tensor(
                h2, sc, 0.5, sig2[pair][:, 128:256],
                ALU.subtract, ALU.mult,
            )
            h_prev[pair] = h2
            y_mm[pair] = (
                slice(pair * 2 * T + 2 * t, pair * 2 * T + 2 * t + 2),
                h2,
            )

    # trailing y matmuls for t = T-1
    for pair in range(NPAIR):
        out_sl, h_tile = y_mm[pair]
        nc.tensor.matmul(
            y_ps[:, out_sl], h_tile, outw[:, 2 * pair : 2 * pair + 2],
            start=True, stop=True,
        )


def _emit_v4(*args, **kw):
    _emit_rotated(*args, split_sigma=False, **kw)


def _emit_v5(*args, **kw):
    _emit_rotated(*args, split_sigma=True, **kw)


def _emit_rotated(nc, tc, mybir, pg_pool, sig_pool, tan_pool, tmp_pool, c_pool,
                  h_pool, xmT, ipw, recw, outw, y_ps, f32, sdt, cdt, SIG, TANH,
                  split_sigma=False):
    """Software-pipelined emission: iteration tau emits the *tail* of step
    tau-1 (s_c, h2) before the matmuls/sigmoid/cell ops of step tau, so each
    engine's in-order queue cycles through both pair-chains with the tail of
    one step overlapping the head of the next.

    Chunk order [i, g, f, o].  split_sigma: sigma1=[i,g,f], sigma2=[o]
    (sigma2 is only needed by h2 one iteration later)."""
    ALU = mybir.AluOpType
    h_prev = [None] * NPAIR
    c_prev = [None] * NPAIR
    o_src = [None] * NPAIR  # AP of sigmoid(o) for the h2 of the previous step
    y_mm = [None] * NPAIR

    for t in range(T + 1):
        # ---- tail of step t-1: s_c, h2 ----
        if t > 0:
            scs = []
            for pair in range(NPAIR):
                sc = tan_pool.tile([128, 128], f32, tag=f"sc{pair}", name=f"sc{pair}")
                nc.scalar.activation(sc, c_prev[pair], SIG, scale=2.0)
                scs.append(sc)
            for pair in range(NPAIR):
                h2 = h_pool.tile([128, B], mybir.dt.bfloat16, tag=f"h{pair}",
                                 name=f"h{pair}")
                nc.vector.scalar_tensor_tensor(
                    h2, scs[pair], 0.5, o_src[pair], ALU.subtract, ALU.mult
                )
                h_prev[pair] = h2
                y_mm[pair] = (
                    slice(pair * 2 * T + 2 * (t - 1), pair * 2 * T + 2 * (t - 1) + 2),
                    h2,
                )
        if t == T:
            break

        xm_t = xmT[:, t * B : (t + 1) * B]

        # ---- gate matmuls for t ----
        pgs = []
        for pair in range(NPAIR):
            pg = pg_pool.tile([128, 4 * 128], f32, tag=f"pg{pair}", name=f"pg{pair}")
            pgs.append(pg)
            for k in range(4):
                ci = pair * 4 + k
                sl = slice(k * 128, (k + 1) * 128)
                wsl = slice(ci * 128, (ci + 1) * 128)
                if t > 0:
                    nc.tensor.matmul(
                        pg[:, sl], recw[:, wsl], h_prev[pair],
                        start=True, stop=False,
                    )
                nc.tensor.matmul(
                    pg[:, sl], ipw[:, wsl], xm_t, start=(t == 0), stop=True
                )
        # y matmuls for t-1
        for pair in range(NPAIR):
            if y_mm[pair] is not None:
                out_sl, h_tile = y_mm[pair]
                nc.tensor.matmul(
                    y_ps[:, out_sl], h_tile, outw[:, 2 * pair : 2 * pair + 2],
                    start=True, stop=True,
                )
                y_mm[pair] = None

        # ---- sigmoid(s) for t ----
        sigs = []
        if split_sigma:
            for pair in range(NPAIR):
                s1 = sig_pool.tile([128, 3 * 128], sdt, tag=f"s1{pair}",
                                   name=f"s1{pair}")
                nc.scalar.activation(s1, pgs[pair][:, 0:384], SIG)
                sigs.append(s1)
            for pair in range(NPAIR):
                s2 = sig_pool.tile([128, 128], sdt, tag=f"s2{pair}",
                                   name=f"s2{pair}")
                nc.scalar.activation(s2, pgs[pair][:, 384:512], SIG)
                o_src[pair] = s2
        else:
            for pair in range(NPAIR):
                s = sig_pool.tile([128, 4 * 128], sdt, tag=f"s{pair}",
                                  name=f"s{pair}")
                nc.scalar.activation(s, pgs[pair], SIG)
                sigs.append(s)
                o_src[pair] = s[:, 384:512]

        # ---- cell update for t: t1' = (s_g-0.5)*s_i ; c = 2*t1' + f*c ----
        for pair in range(NPAIR):
            s = sigs[pair]
            t1 = tmp_pool.tile([128, 128], sdt, tag=f"t1{pair}", name=f"t1{pair}")
            nc.vector.scalar_tensor_tensor(
                t1, s[:, 128:256], 0.5, s[:, 0:128], ALU.subtract, ALU.mult
            )
            c_new = c_pool.tile([128, 128], cdt, tag=f"c{pair}", name=f"c{pair}")
            if t == 0:
                nc.vector.tensor_scalar_mul(c_new, t1, 2.0)
            else:
                t2 = tmp_pool.tile([128, 128], cdt, tag=f"t2{pair}", name=f"t2{pair}")
                eng = nc.gpsimd if USE_GPSIMD else nc.vector
                eng.tensor_mul(t2, s[:, 256:384], c_prev[pair])
                nc.vector.scalar_tensor_tensor(
                    c_new, t1, 2.0, t2, ALU.mult, ALU.add
                )
            c_prev[pair] = c_new

    # trailing y matmuls for t = T-1
    for pair in range(NPAIR):
        out_sl, h_tile = y_mm[pair]
        nc.tensor.matmul(
            y_ps[:, out_sl], h_tile, outw[:, 2 * pair : 2 * pair + 2],
            start=True, stop=True,
        )


def _emit_v7(nc, tc, mybir, pg_pool, sig_pool, tan_pool, tmp_pool, c_pool,
             h_pool, xmT, ipw, recw, outw, y_ps, f32, sdt, cdt, SIG, TANH):
    """v5 chain ops, but emitted as complete per-pair blocks so each
    engine's in-order queue alternates whole chain-stages of the two pairs
    (anti-phase) instead of interleaving the same stage of both pairs."""
    ALU = mybir.AluOpType
    h_prev = [None] * NPAIR
    c_prev = [None] * NPAIR
    o_src = [None] * NPAIR
    y_mm = [None] * NPAIR

    for t in range(T + 1):
        for pair in range(NPAIR):
            # ---- tail of step t-1 for this pair ----
            if t > 0:
                sc = tan_pool.tile([128, 128], f32, tag=f"sc{pair}", name=f"sc{pair}")
                nc.scalar.activation(sc, c_prev[pair], SIG, scale=2.0)
                h2 = h_pool.tile([128, B], mybir.dt.bfloat16, tag=f"h{pair}",
                                 name=f"h{pair}")
                nc.vector.scalar_tensor_tensor(
                    h2, sc, 0.5, o_src[pair], ALU.subtract, ALU.mult
                )
                h_prev[pair] = h2
            if t == T:
                continue

            xm_t = xmT[:, t * B : (t + 1) * B]
            pg = pg_pool.tile([128, 4 * 128], f32, tag=f"pg{pair}", name=f"pg{pair}")
            for k in range(4):
                ci = pair * 4 + k
                sl = slice(k * 128, (k + 1) * 128)
                wsl = slice(ci * 128, (ci + 1) * 128)
                if t > 0:
                    nc.tensor.matmul(pg[:, sl], recw[:, wsl], h_prev[pair],
                                     start=True, stop=False)
                nc.tensor.matmul(pg[:, sl], ipw[:, wsl], xm_t,
                                 start=(t == 0), stop=True)
            # y matmul for t-1 of this pair
            if y_mm[pair] is not None:
                out_sl, h_tile = y_mm[pair]
                nc.tensor.matmul(
                    y_ps[:, out_sl], h_tile, outw[:, 2 * pair : 2 * pair + 2],
                    start=True, stop=True,
                )
            if t > 0:
                y_mm[pair] = (
                    slice(pair * 2 * T + 2 * (t - 1), pair * 2 * T + 2 * (t - 1) + 2),
                    h_prev[pair],
                )

            # sigma1 = [i, g, f]; sigma2 = [o]
            s1 = sig_pool.tile([128, 3 * 128], sdt, tag=f"s1{pair}", name=f"s1{pair}")
            nc.scalar.activation(s1, pg[:, 0:384], SIG)
            s2 = sig_pool.tile([128, 128], sdt, tag=f"s2{pair}", name=f"s2{pair}")
            nc.scalar.activation(s2, pg[:, 384:512], SIG)
            o_src[pair] = s2

            # cell update
            t1 = tmp_pool.tile([128, 128], sdt, tag=f"t1{pair}", name=f"t1{pair}")
            nc.vector.scalar_tensor_tensor(
                t1, s1[:, 128:256], 0.5, s1[:, 0:128], ALU.subtract, ALU.mult
            )
            c_new = c_pool.tile([128, 128], cdt, tag=f"c{pair}", name=f"c{pair}")
            if t == 0:
                nc.vector.tensor_scalar_mul(c_new, t1, 2.0)
            else:
                t2 = tmp_pool.tile([128, 128], cdt, tag=f"t2{pair}", name=f"t2{pair}")
                nc.vector.tensor_mul(t2, s1[:, 256:384], c_prev[pair])
                nc.vector.scalar_tensor_tensor(
                    c_new, t1, 2.0, t2, ALU.mult, ALU.add
                )
            c_prev[pair] = c_new

    # trailing y matmuls: the still-pending (T-2) and the final (T-1)
    for pair in range(NPAIR):
        if y_mm[pair] is not None:
            out_sl, h_tile = y_mm[pair]
            nc.tensor.matmul(
                y_ps[:, out_sl], h_tile, outw[:, 2 * pair : 2 * pair + 2],
                start=True, stop=True,
            )
        nc.tensor.matmul(
            y_ps[:, pair * 2 * T + 2 * (T - 1) : pair * 2 * T + 2 * (T - 1) + 2],
            h_prev[pair], outw[:, 2 * pair : 2 * pair + 2],
            start=True, stop=True,
        )


def _emit_v8(nc, tc, mybir, pga_pool, pgb_pool, sig_pool, tan_pool, tmp_pool,
             c_pool, h_pool, xmT, ipw, recw, outw, y_ps, f32, sdt, cdt,
             SIG, TANH):
    """v5 + bank split matching the sigma split: pga=[i,g,f] (bufs=2, the
    sigma1 bank -- sigma1 now waits only 6 matmuls), pgb=[o] (bufs=1,
    sigma2 is fully off the critical path)."""
    ALU = mybir.AluOpType
    h_prev = [None] * NPAIR
    c_prev = [None] * NPAIR
    o_src = [None] * NPAIR
    y_mm = [None] * NPAIR

    for t in range(T + 1):
        if t > 0:
            scs = []
            for pair in range(NPAIR):
                sc = tan_pool.tile([128, 128], f32, tag=f"sc{pair}", name=f"sc{pair}")
                nc.scalar.activation(sc, c_prev[pair], SIG, scale=2.0)
                scs.append(sc)
            for pair in range(NPAIR):
                h2 = h_pool.tile([128, B], mybir.dt.bfloat16, tag=f"h{pair}",
                                 name=f"h{pair}")
                nc.vector.scalar_tensor_tensor(
                    h2, scs[pair], 0.5, o_src[pair], ALU.subtract, ALU.mult
                )
                h_prev[pair] = h2
                y_mm[pair] = (
                    slice(pair * 2 * T + 2 * (t - 1), pair * 2 * T + 2 * (t - 1) + 2),
                    h2,
                )
        if t == T:
            break

        xm_t = xmT[:, t * B : (t + 1) * B]

        # [i, g, f] chunks into pga (both pairs), then [o] into pgb
        pgas, pgbs = [], []
        for pair in range(NPAIR):
            pga = pga_pool.tile([128, 3 * 128], f32, tag=f"pga{pair}",
                                name=f"pga{pair}")
            pgas.append(pga)
            for k in range(3):
                ci = pair * 4 + k
                sl = slice(k * 128, (k + 1) * 128)
                wsl = slice(ci * 128, (ci + 1) * 128)
                if t > 0:
                    nc.tensor.matmul(pga[:, sl], recw[:, wsl], h_prev[pair],
                                     start=True, stop=False)
                nc.tensor.matmul(pga[:, sl], ipw[:, wsl], xm_t,
                                 start=(t == 0), stop=True)
        for pair in range(NPAIR):
            pgb = pgb_pool.tile([128, 128], f32, tag=f"pgb{pair}",
                                name=f"pgb{pair}")
            pgbs.append(pgb)
            ci = pair * 4 + 3
            wsl = slice(ci * 128, (ci + 1) * 128)
            if t > 0:
                nc.tensor.matmul(pgb, recw[:, wsl], h_prev[pair],
                                 start=True, stop=False)
            nc.tensor.matmul(pgb, ipw[:, wsl], xm_t,
                             start=(t == 0), stop=True)
        for pair in range(NPAIR):
            if y_mm[pair] is not None:
                out_sl, h_tile = y_mm[pair]
                nc.tensor.matmul(
                    y_ps[:, out_sl], h_tile, outw[:, 2 * pair : 2 * pair + 2],
                    start=True, stop=True,
                )
                y_mm[pair] = None

        # sigma1 = [i, g, f] (chain); sigma2 = [o] (off-chain)
        s1s = []
        for pair in range(NPAIR):
            s1 = sig_pool.tile([128, 3 * 128], sdt, tag=f"s1{pair}", name=f"s1{pair}")
            nc.scalar.activation(s1, pgas[pair], SIG)
            s1s.append(s1)
        for pair in range(NPAIR):
            s2 = sig_pool.tile([128, 128], sdt, tag=f"s2{pair}", name=f"s2{pair}")
            nc.scalar.activation(s2, pgbs[pair], SIG)
            o_src[pair] = s2

        for pair in range(NPAIR):
            s1 = s1s[pair]
            t1 = tmp_pool.tile([128, 128], sdt, tag=f"t1{pair}", name=f"t1{pair}")
            nc.vector.scalar_tensor_tensor(
                t1, s1[:, 128:256], 0.5, s1[:, 0:128], ALU.subtract, ALU.mult
            )
            c_new = c_pool.tile([128, 128], cdt, tag=f"c{pair}", name=f"c{pair}")
            if t == 0:
                nc.vector.tensor_scalar_mul(c_new, t1, 2.0)
            else:
                t2 = tmp_pool.tile([128, 128], cdt, tag=f"t2{pair}", name=f"t2{pair}")
                nc.vector.tensor_mul(t2, s1[:, 256:384], c_prev[pair])
                nc.vector.scalar_tensor_tensor(
                    c_new, t1, 2.0, t2, ALU.mult, ALU.add
                )
            c_prev[pair] = c_new

    for pair in range(NPAIR):
        out_sl, h_tile = y_mm[pair]
        nc.tensor.matmul(
            y_ps[:, out_sl], h_tile, outw[:, 2 * pair : 2 * pair + 2],
            start=True, stop=True,
        )


def _emit_v6(nc, tc, mybir, pga_pool, pgb_pool, sig_pool, tan_pool, tmp_pool,
             c_pool, h_pool, xmT, ipw, recw, outw, y_ps, f32, sdt, cdt,
             SIG, TANH):
    """v5 + two-bank psum split per pair: pga=[i,g] (bufs=2), pgb=[f,o]
    (bufs=1), so sigma1 only waits on 4 matmuls and is smaller."""
    ALU = mybir.AluOpType
    h_prev = [None] * NPAIR
    c_prev = [None] * NPAIR
    o_src = [None] * NPAIR
    y_mm = [None] * NPAIR

    for t in range(T + 1):
        if t > 0:
            scs = []
            for pair in range(NPAIR):
                sc = tan_pool.tile([128, 128], f32, tag=f"sc{pair}", name=f"sc{pair}")
                nc.scalar.activation(sc, c_prev[pair], SIG, scale=2.0)
                scs.append(sc)
            for pair in range(NPAIR):
                h2 = h_pool.tile([128, B], mybir.dt.bfloat16, tag=f"h{pair}",
                                 name=f"h{pair}")
                nc.vector.scalar_tensor_tensor(
                    h2, scs[pair], 0.5, o_src[pair], ALU.subtract, ALU.mult
                )
                h_prev[pair] = h2
                y_mm[pair] = (
                    slice(pair * 2 * T + 2 * (t - 1), pair * 2 * T + 2 * (t - 1) + 2),
                    h2,
                )
        if t == T:
            break

        xm_t = xmT[:, t * B : (t + 1) * B]

        # gate matmuls: [i,g] chunks (pga) for both pairs first, then [f,o]
        pgas, pgbs = [], []
        for pair in range(NPAIR):
            pga = pga_pool.tile([128, 2 * 128], f32, tag=f"pga{pair}",
                                name=f"pga{pair}")
            pgas.append(pga)
            for k in range(2):
                ci = pair * 4 + k
                sl = slice(k * 128, (k + 1) * 128)
                wsl = slice(ci * 128, (ci + 1) * 128)
                if t > 0:
                    nc.tensor.matmul(pga[:, sl], recw[:, wsl], h_prev[pair],
                                     start=True, stop=False)
                nc.tensor.matmul(pga[:, sl], ipw[:, wsl], xm_t,
                                 start=(t == 0), stop=True)
        for pair in range(NPAIR):
            pgb = pgb_pool.tile([128, 2 * 128], f32, tag=f"pgb{pair}",
                                name=f"pgb{pair}")
            pgbs.append(pgb)
            for k in range(2, 4):
                ci = pair * 4 + k
                sl = slice((k - 2) * 128, (k - 1) * 128)
                wsl = slice(ci * 128, (ci + 1) * 128)
                if t > 0:
                    nc.tensor.matmul(pgb[:, sl], recw[:, wsl], h_prev[pair],
                                     start=True, stop=False)
                nc.tensor.matmul(pgb[:, sl], ipw[:, wsl], xm_t,
                                 start=(t == 0), stop=True)
        for pair in range(NPAIR):
            if y_mm[pair] is not None:
                out_sl, h_tile = y_mm[pair]
                nc.tensor.matmul(
                    y_ps[:, out_sl], h_tile, outw[:, 2 * pair : 2 * pair + 2],
                    start=True, stop=True,
                )
                y_mm[pair] = None

        # sigma1 = [i, g]; sigma2 = [f, o]
        s1s, s2s = [], []
        for pair in range(NPAIR):
            s1 = sig_pool.tile([128, 2 * 128], sdt, tag=f"s1{pair}",
                               name=f"s1{pair}")
            nc.scalar.activation(s1, pgas[pair], SIG)
            s1s.append(s1)
        for pair in range(NPAIR):
            s2 = sig_pool.tile([128, 2 * 128], sdt, tag=f"s2{pair}",
                               name=f"s2{pair}")
            nc.scalar.activation(s2, pgbs[pair], SIG)
            s2s.append(s2)
            o_src[pair] = s2[:, 128:256]

        # t1' = (s_g - 0.5)*s_i ; c = 2*t1' + s_f*c_prev
        for pair in range(NPAIR):
            s1 = s1s[pair]
            t1 = tmp_pool.tile([128, 128], sdt, tag=f"t1{pair}", name=f"t1{pair}")
            nc.vector.scalar_tensor_tensor(
                t1, s1[:, 128:256], 0.5, s1[:, 0:128], ALU.subtract, ALU.mult
            )
            c_new = c_pool.tile([128, 128], cdt, tag=f"c{pair}", name=f"c{pair}")
            if t == 0:
                nc.vector.tensor_scalar_mul(c_new, t1, 2.0)
            else:
                t2 = tmp_pool.tile([128, 128], cdt, tag=f"t2{pair}", name=f"t2{pair}")
                nc.vector.tensor_mul(t2, s2s[pair][:, 0:128], c_prev[pair])
                nc.vector.scalar_tensor_tensor(
                    c_new, t1, 2.0, t2, ALU.mult, ALU.add
                )
            c_prev[pair] = c_new

    for pair in range(NPAIR):
        out_sl, h_tile = y_mm[pair]
        nc.tensor.matmul(
            y_ps[:, out_sl], h_tile, outw[:, 2 * pair : 2 * pair + 2],
            start=True, stop=True,
        )


def _w_full(W_ih_d, d):
    """[4H, D-1] -> [4H, D] with column d zero and the 'other feature'
    columns scattered back to their true feature index."""
    out = np.zeros((4 * H, D), np.float32)
    idx = [j for j in range(D) if j != d]
    out[:, idx] = W_ih_d
    return out


def _pack_core_inputs(core, xmT_np, W_ih, W_hh, b_ih, b_hh, W_out):
    """Pack weights for one core (features 4*core .. 4*core+3)."""
    ipw = np.zeros((D + 1, 8 * 128), np.float32)
    recw = np.zeros((128, 8 * 128), np.float32)
    outw = np.zeros((128, 2 * NPAIR), np.float32)

    for ci, (gate, pair) in enumerate(CHUNK_DEFS):
        gs = GATE_SLICES[gate]
        for half in range(2):
            d = DLOC * core + 2 * pair + half
            cols = slice(ci * 128 + 64 * half, ci * 128 + 64 * half + 64)
            rows = slice(64 * half, 64 * half + 64)
            # rec: block-diag W_hh[d, gate_rows, :].T  ([K=h, M=gate_row])
            recw[rows, cols] = W_hh[d, gs, :].T
            # ip: full-D input weights with zero self-column, bias in row 32
            wf = _w_full(W_ih[d], d)  # [4H, D]
            ipw[0:D, cols] = wf[gs, :].T
            ipw[D, cols] = b_ih[d, gs] + b_hh[d, gs]

    for pair in range(NPAIR):
        for half in range(2):
            d = DLOC * core + 2 * pair + half
            outw[64 * half : 64 * half + 64, 2 * pair + half] = W_out[d]

    if V3:
        # sigma-trick folding: h is stored as h/2 -> double recw/outw;
        # g-gate logits doubled -> double g chunks of ipw and recw again.
        recw *= 2.0
        outw *= 2.0
        for ci, (gate, _) in enumerate(CHUNK_DEFS):
            if gate == "g":
                ipw[:, ci * 128 : (ci + 1) * 128] *= 2.0
                recw[:, ci * 128 : (ci + 1) * 128] *= 2.0

    return {
        "xmT": xmT_np,
        "ipw": ipw.astype(BF16),
        "recw": recw.astype(BF16),
        "outw": outw.astype(BF16),
    }


def _prep_in_maps(x_raw, mask_pad, W_ih, W_hh, b_ih, b_hh, W_out):
    xm = np.where(mask_pad[:, :, None], x_raw, 0.0).astype(np.float32)  # [B,T,D]
    xmT = np.empty((D + 1, T * B), np.float32)
    xmT[0:D] = xm.transpose(2, 1, 0).reshape(D, T * B)  # [d, t*B + b]
    xmT[D] = 1.0
    xmT_np = xmT.astype(BF16)
    return [
        _pack_core_inputs(k, xmT_np, W_ih, W_hh, b_ih, b_hh, W_out)
        for k in range(NCORES)
    ]


def _assemble_output(results, b_out):
    """results[k]["y"]: [B, NPAIR*2*T] fp32, layout [b, pair*2T + t*2 + half]."""
    x_hat = np.empty((B, T, D), np.float32)
    for k in range(NCORES):
        y = np.asarray(results[k]["y"]).reshape(B, NPAIR, T, 2)
        for pair in range(NPAIR):
            for half in range(2):
                d = DLOC * k + 2 * pair + half
                x_hat[:, :, d] = y[:, pair, :, half] + b_out[d]
    return x_hat


def kernel(x_raw, mask_pad, W_ih, W_hh, b_ih, b_hh, W_out, b_out):
    x_raw = np.asarray(x_raw, np.float32)
    mask_pad = np.asarray(mask_pad)
    W_ih = np.asarray(W_ih, np.float32)
    W_hh = np.asarray(W_hh, np.float32)
    b_ih = np.asarray(b_ih, np.float32)
    b_hh = np.asarray(b_hh, np.float32)
    W_out = np.asarray(W_out, np.float32)
    b_out = np.asarray(b_out, np.float32)

    from concourse import bass_utils

    nc = _build_bass()
    in_maps = _prep_in_maps(x_raw, mask_pad, W_ih, W_hh, b_ih, b_hh, W_out)
    res = bass_utils.run_bass_kernel_spmd(
        nc,
        in_maps,
        core_ids=list(range(NCORES)),
        trace=bool(int(os.environ.get("KERNEL_TRACE", "0"))),
    )
    _CACHE["last_results"] = res
    return _assemble_output(res.results, b_out)
